# revision 22
# baseline (speedup 1.0000x reference)
"""Multi-head attention (B=4, S=2048, D=1024, H=16, DK=64) on 8 Trainium2
NeuronCores.

v3: head-sharding.  Core c = 2*b + j handles batch b = c//2 and HEADS
j*8..j*8+7 over the FULL query range (v1/v2 split queries, which made
both cores of a batch recompute full-S K/V — 131k duplicated PE-cycles
per core).  Each core now projects only its 8 heads' K/V/Q columns and
emits a PARTIAL output (its heads' ctx @ its 512 Wo rows); the host
sums the two partials per batch when gathering.  No collectives.

PE tile-packing as v2: the two heads of a pair run concurrently on
disjoint array quadrants (scores on row groups, PV ctx on col groups,
4-way col-packed [1,512] denominator matmuls at partitions 0/32/64/96
of one PSUM bank).  E^T = exp(scores^T/8 + mask - 3) streamed per
(pair, chunk); K/Q projection chains for the next head-pair drip-fed
as PE filler.  bv/bo fold into radd = bv @ Wo + bo host-side (applied
on the j=0 core only).
"""

import itertools

import numpy as np
import ml_dtypes

B, S, D, H, DK = 4, 2048, 1024, 16, 64
N_CORES = 8
HL = 8               # local heads per core
HP = HL // 2         # local head-pairs
NEG_C = -3.0         # exp stabilizer; cancels exactly in normalization
BF = ml_dtypes.bfloat16


def _build():
    import concourse.mybir as mybir
    import concourse.tile as tile
    from concourse import bacc

    dt = mybir.dt
    AF = mybir.ActivationFunctionType
    nc = bacc.Bacc("TRN2", num_devices=N_CORES)

    W = HL * DK      # 512: this core's projection width
    xt = nc.declare_dram_parameter("xt", [D, S], dt.bfloat16, isOutput=False)
    wq = nc.declare_dram_parameter("wq", [D, W], dt.bfloat16, isOutput=False)
    wk = nc.declare_dram_parameter("wk", [D, W], dt.bfloat16, isOutput=False)
    wv = nc.declare_dram_parameter("wv", [D, W], dt.bfloat16, isOutput=False)
    wo = nc.declare_dram_parameter("wo", [W, D], dt.bfloat16, isOutput=False)
    bq = nc.declare_dram_parameter("bq", [W], dt.float32, isOutput=False)
    bk = nc.declare_dram_parameter("bk", [W], dt.float32, isOutput=False)
    radd = nc.declare_dram_parameter("radd", [D], dt.float32, isOutput=False)
    mk = nc.declare_dram_parameter("mk", [S], dt.float32, isOutput=False)
    out = nc.declare_dram_parameter("out", [S, D], dt.float32, isOutput=True)

    with tile.TileContext(nc) as tc:
        with (
            tc.tile_pool(name="pers", bufs=1) as pers,
            tc.tile_pool(name="psS", bufs=2, space="PSUM") as psS,
            tc.tile_pool(name="psF", bufs=1, space="PSUM") as psF,
            tc.tile_pool(name="ctxp", bufs=2, space="PSUM") as ctxp,
            tc.tile_pool(name="denp", bufs=1, space="PSUM") as denp,
            tc.tile_pool(name="ktp", bufs=2) as ktp,
            tc.tile_pool(name="qtp", bufs=2) as qtp,
            tc.tile_pool(name="wstr", bufs=2) as wstr,
        ):
            # ---- persistent SBUF arrays -------------------------------
            v_s = pers.tile([128, 16 * W], dt.bfloat16, tag="v")
            xt_s = pers.tile([128, 8 * S], dt.bfloat16, tag="xt")
            wk_s = pers.tile([128, 8 * W], dt.bfloat16, tag="wk")
            bqc = pers.tile([128, HP], dt.float32, tag="bqc")
            bkc = pers.tile([128, HP], dt.float32, tag="bkc")
            mkc = pers.tile([128, 16], dt.float32, tag="mkc")
            ones_s = pers.tile([128, 32], dt.bfloat16, tag="ones")

            nc.sync.dma_start(out=bqc, in_=bq.rearrange("(a p) -> p a", p=128))
            nc.sync.dma_start(out=bkc, in_=bk.rearrange("(a p) -> p a", p=128))
            nc.sync.dma_start(out=mkc, in_=mk.rearrange("(a p) -> p a", p=128))
            nc.gpsimd.memset(ones_s, 1.0)

            # ---- phase 1: V (wv freed after) --------------------------
            with tc.tile_pool(name="poolA", bufs=1) as poolA:
                wv_s = poolA.tile([128, 8 * W], dt.bfloat16, tag="wv")
                for c in range(8):
                    nc.sync.dma_start(
                        out=xt_s[:, c * S:(c + 1) * S],
                        in_=xt[c * 128:(c + 1) * 128, :])
                    nc.sync.dma_start(
                        out=wv_s[:, c * W:(c + 1) * W],
                        in_=wv[c * 128:(c + 1) * 128, :])
                for c in range(8):
                    nc.sync.dma_start(
                        out=wk_s[:, c * W:(c + 1) * W],
                        in_=wk[c * 128:(c + 1) * 128, :])

                for sc2 in range(8):
                    pv = psS.tile([128, 1024], dt.float32, tag="sp",
                                  name=f"pv{sc2}")
                    for half in range(2):
                        sc = 2 * sc2 + half
                        for c in range(8):
                            nc.tensor.matmul(
                                out=pv[:, half * 512:(half + 1) * 512],
                                lhsT=xt_s[:, c * S + sc * 128:
                                          c * S + (sc + 1) * 128],
                                rhs=wv_s[:, c * W:(c + 1) * W],
                                start=(c == 0), stop=(c == 7))
                    # pv columns are (h, dk) pairs == the V layout we want
                    nc.vector.tensor_copy(
                        v_s[:, sc2 * 1024:(sc2 + 1) * 1024], pv)

            # ---- phase 2: interleaved projections + attention ---------
            with (
                tc.tile_pool(name="attin", bufs=1) as attin,
                tc.tile_pool(name="epool", bufs=10) as epool,
                tc.tile_pool(name="rcpp", bufs=5) as rcpp,
                tc.tile_pool(name="rpool", bufs=3) as rpool,
                tc.tile_pool(name="stg", bufs=2) as stg,
                tc.tile_pool(name="opool", bufs=2) as opool,
            ):
                ctxt_s = attin.tile([128, HP * S], dt.bfloat16, tag="ctxt")
                wo_s = attin.tile([128, HP * D], dt.bfloat16, tag="wo")
                bob = attin.tile([128, D], dt.float32, tag="bob")
                for r in range(HP):
                    nc.sync.dma_start(
                        out=wo_s[:, r * D:(r + 1) * D],
                        in_=wo[r * 128:(r + 1) * 128, :])

                def _bcast_src(ap):
                    import concourse.bass as bass
                    return bass.AP(
                        tensor=ap.tensor, offset=ap.offset,
                        ap=[[0, 128]] + [list(p) for p in ap.ap])

                nc.gpsimd.dma_start(out=bob, in_=_bcast_src(radd[:]))

                kt_tiles = {}
                qt_tiles = {}

                # Filler-unit generators: projection chains for head-pair
                # `hp`, emitted in small steps interleaved with attention.
                def k_chain_units(hp):
                    kt_t = ktp.tile([128, S], dt.bfloat16, tag="ktt",
                                    name=f"ktt{hp}")
                    kt_tiles[hp] = kt_t
                    for t in range(4):
                        pk = psF.tile([128, 512], dt.float32, tag="pf",
                                      name=f"pk{hp}_{t}")
                        for c in range(8):
                            def do_k(c=c, t=t, pk=pk):
                                nc.tensor.matmul(
                                    out=pk,
                                    lhsT=wk_s[:, c * W + hp * 128:
                                              c * W + (hp + 1) * 128],
                                    rhs=xt_s[:, c * S + t * 512:
                                             c * S + (t + 1) * 512],
                                    start=(c == 0), stop=(c == 7))
                            yield do_k
                        def drain_k(t=t, pk=pk, kt_t=kt_t):
                            nc.vector.tensor_scalar_add(
                                kt_t[:, t * 512:(t + 1) * 512],
                                pk, bkc[:, hp:hp + 1])
                        yield drain_k

                def q_chain_units(hp):
                    qt_t = qtp.tile([128, S], dt.bfloat16, tag="qtt",
                                    name=f"qtt{hp}")
                    qt_tiles[hp] = qt_t
                    wqc = wstr.tile([128, 1024], dt.bfloat16, tag="wqs",
                                    name=f"wqc{hp}")
                    nc.sync.dma_start(
                        out=wqc.rearrange("p (c n) -> p c n", n=128),
                        in_=wq.rearrange("(c p) n -> p c n", p=128)[
                            :, :, hp * 128:(hp + 1) * 128])
                    for t in range(4):
                        pq = psF.tile([128, 512], dt.float32, tag="pf",
                                      name=f"pq{hp}_{t}")
                        for c in range(8):
                            def do_q(c=c, t=t, pq=pq, wqc=wqc):
                                nc.tensor.matmul(
                                    out=pq,
                                    lhsT=wqc[:, c * 128:(c + 1) * 128],
                                    rhs=xt_s[:, c * S + t * 512:
                                             c * S + (t + 1) * 512],
                                    start=(c == 0), stop=(c == 7))
                            yield do_q
                        def drain_q(t=t, pq=pq, qt_t=qt_t):
                            nc.vector.tensor_scalar_add(
                                qt_t[:, t * 512:(t + 1) * 512],
                                pq, bqc[:, hp:hp + 1])
                        yield drain_q

                def drive(gen, n=1):
                    if gen is None:
                        return
                    for _ in range(n):
                        for u in gen:
                            u()
                            break
                        else:
                            return

                def finish(gen):
                    if gen is not None:
                        for u in gen:
                            u()

                # prime head-pair 0 (wide PSUM tiles from psS — the
                # quarter-granularity psF bank would bubble on drains
                # with no attention work interleaved yet)
                kt_t = ktp.tile([128, S], dt.bfloat16, tag="ktt",
                                name="ktt0")
                kt_tiles[0] = kt_t
                for hf in range(2):
                    pk = psS.tile([128, 1024], dt.float32, tag="sp",
                                  name=f"Ppk{hf}")
                    for c in range(8):
                        lhsT = wk_s[:, c * W:c * W + 128]
                        for st2 in range(2):
                            nc.tensor.matmul(
                                out=pk[:, st2 * 512:(st2 + 1) * 512],
                                lhsT=lhsT,
                                rhs=xt_s[:, c * S + hf * 1024 + st2 * 512:
                                         c * S + hf * 1024 + (st2 + 1) * 512],
                                start=(c == 0), stop=(c == 7))
                    nc.vector.tensor_scalar_add(
                        kt_t[:, hf * 1024:(hf + 1) * 1024],
                        pk, bkc[:, 0:1])
                qt_t = qtp.tile([128, S], dt.bfloat16, tag="qtt",
                                name="qtt0")
                qt_tiles[0] = qt_t
                wqc = wstr.tile([128, 1024], dt.bfloat16, tag="wqs",
                                name="wqc0")
                nc.sync.dma_start(
                    out=wqc.rearrange("p (c n) -> p c n", n=128),
                    in_=wq.rearrange("(c p) n -> p c n", p=128)[:, :, 0:128])
                for hf in range(2):
                    pq = psS.tile([128, 1024], dt.float32, tag="sp",
                                  name=f"Ppq{hf}")
                    for c in range(8):
                        lhsT = wqc[:, c * 128:(c + 1) * 128]
                        for st2 in range(2):
                            nc.tensor.matmul(
                                out=pq[:, st2 * 512:(st2 + 1) * 512],
                                lhsT=lhsT,
                                rhs=xt_s[:, c * S + hf * 1024 + st2 * 512:
                                         c * S + hf * 1024 + (st2 + 1) * 512],
                                start=(c == 0), stop=(c == 7))
                    nc.vector.tensor_scalar_add(
                        qt_t[:, hf * 1024:(hf + 1) * 1024],
                        pq, bqc[:, 0:1])

                fillers = {
                    hp: itertools.chain(k_chain_units(hp), q_chain_units(hp))
                    for hp in range(1, HP)
                }

                DEN_LAG = 3  # see den-bank release note below
                pending_norm = []

                for vp in range(2 * HP):
                    hp, jq = vp // 2, vp % 2
                    filler = fillers.get(vp // 2 + 1)
                    kt_t, qt_t = kt_tiles[hp], qt_tiles[hp]
                    hA, hB = 2 * hp, 2 * hp + 1
                    qof = jq * 1024
                    # one PSUM bank per q-half holds BOTH heads' ctx
                    cx = [ctxp.tile([128, 512], dt.float32, tag="cx",
                                    name=f"cx{vp}_{q2}") for q2 in range(2)]
                    # one bank holds the 4 denominator accumulators at
                    # partitions 0/32/64/96 (q2*64 + head*32)
                    dn = denp.tile([128, 512], dt.float32, tag="dn",
                                   name=f"dn{vp}")
                    e_hist = {}
                    for g in range(16 + DEN_LAG):
                        if g < 16:
                            scc = g
                            spA = psS.tile([128, 1024], dt.float32, tag="sp",
                                           name=f"spA{vp}_{scc}")
                            spB = psS.tile([128, 1024], dt.float32, tag="sp",
                                           name=f"spB{vp}_{scc}")
                            for po, sp in ((0, spA), (64, spB)):
                                lhsT = kt_t[po:po + 64,
                                            scc * 128:(scc + 1) * 128]
                                for q2 in range(2):
                                    nc.tensor.matmul(
                                        out=sp[:, q2 * 512:(q2 + 1) * 512],
                                        lhsT=lhsT,
                                        rhs=qt_t[po:po + 64,
                                                 qof + q2 * 512:
                                                 qof + (q2 + 1) * 512],
                                        start=True, stop=True)
                            eA = epool.tile([128, 1024], dt.bfloat16,
                                            tag="e", name=f"eA{vp}_{scc}")
                            eB = epool.tile([128, 1024], dt.bfloat16,
                                            tag="e", name=f"eB{vp}_{scc}")
                            nc.scalar.activation(
                                out=eA, in_=spA, func=AF.Exp,
                                bias=mkc[:, scc:scc + 1],
                                scale=1.0 / np.sqrt(DK))
                            nc.scalar.activation(
                                out=eB, in_=spB, func=AF.Exp,
                                bias=mkc[:, scc:scc + 1],
                                scale=1.0 / np.sqrt(DK))
                            e_hist[scc] = (eA, eB)
                        if 1 <= g <= 16:
                            scc = g - 1
                            eA, eB = e_hist[scc]
                            st_, sp_ = (scc == 0), (scc == 15)
                            for q2 in range(2):
                                for po, h, e_ in ((0, hA, eA), (64, hB, eB)):
                                    vh = v_s[:, scc * W + h * DK:
                                             scc * W + (h + 1) * DK]
                                    nc.tensor.matmul(
                                        out=cx[q2][po:po + 64, :], lhsT=vh,
                                        rhs=e_[:, q2 * 512:(q2 + 1) * 512],
                                        start=st_, stop=sp_,
                                        skip_group_check=True)
                        # den + filler emitted in 2-chunk blocks (every
                        # other g) -- fewer PE tiling-mode switches
                        if DEN_LAG <= g and (g % 2 == 1 or g == 16 + DEN_LAG - 1):
                            den_sccs = [s_ for s_ in range(16)
                                        if s_ <= g - DEN_LAG and s_ in e_hist]
                            for scc in den_sccs:
                                eA, eB = e_hist.pop(scc)
                                st_, sp_ = (scc == 0), (scc == 15)
                                for q2 in range(2):
                                    for hh, e_ in ((0, eA), (1, eB)):
                                        p = q2 * 64 + hh * 32
                                        nc.tensor.matmul(
                                            out=dn[p:p + 32, :], lhsT=ones_s,
                                            rhs=e_[:, q2 * 512:(q2 + 1) * 512],
                                            start=st_, stop=sp_,
                                            tile_position=(0, p),
                                            skip_group_check=True)
                        if pending_norm:
                            for u in pending_norm[0]:
                                u()
                                break
                            else:
                                pending_norm.pop(0)
                        if g % 2 == 0:
                            drive(filler, 4)
                    if vp % 2 == 1:
                        finish(filler)

                    # drain ctx + den: fast copies PSUM->SBUF free the
                    # banks within ~1us of the last matmul.  Head B's rows
                    # move from PSUM partitions 64-127 to a base-0 tile so
                    # the muls see matching input base partitions.
                    stA = stg.tile([64, 1024], dt.bfloat16, tag="stA",
                                   name=f"stA{vp}")
                    stB = stg.tile([64, 1024], dt.bfloat16, tag="stB",
                                   name=f"stB{vp}")
                    for q2 in range(2):
                        nc.vector.tensor_copy(
                            stA[:, q2 * 512:(q2 + 1) * 512], cx[q2][0:64, :])
                        nc.vector.tensor_copy(
                            stB[:, q2 * 512:(q2 + 1) * 512], cx[q2][64:128, :])
                    dnc = stg.tile([128, 512], dt.float32, tag="dnc",
                                   name=f"dnc{vp}")
                    nc.vector.tensor_copy(dnc, dn)

                    # the slow normalize chain (one merged reciprocal,
                    # per-accumulator source rows, broadcasts, muls) is
                    # deferred into the next pair's g-steps so it never
                    # blocks filler drains in the DVE FIFO
                    def norm_units(vp=vp, hp=hp, qof=qof,
                                   stA=stA, stB=stB, dnc=dnc):
                        rcp_s = rcpp.tile([128, 512], dt.bfloat16,
                                          tag="rcp", name=f"rcp{vp}")
                        def u_recip():
                            with nc.allow_low_precision(
                                    reason="softmax denom ~1e2, bf16 ok"):
                                nc.vector.reciprocal(out=rcp_s, in_=dnc)
                        yield u_recip
                        srcs = {}
                        for q2 in range(2):
                            for hh in range(2):
                                p = q2 * 64 + hh * 32
                                r = rcpp.tile([1, 512], dt.bfloat16,
                                              tag="rsrc",
                                              name=f"rsrc{vp}_{q2}_{hh}")
                                srcs[(q2, hh)] = r
                                def u_src(r=r, p=p):
                                    nc.vector.tensor_copy(
                                        r, rcp_s[p:p + 1, :])
                                yield u_src
                        for hh, st_h in ((0, stA), (1, stB)):
                            rb = rpool.tile([64, 1024], dt.bfloat16,
                                            tag="rb", name=f"rb{vp}_{hh}")
                            def u_norm(hh=hh, st_h=st_h, rb=rb):
                                for q2 in range(2):
                                    nc.gpsimd.partition_broadcast(
                                        rb[:, q2 * 512:(q2 + 1) * 512],
                                        srcs[(q2, hh)][0:1, :])
                                nc.vector.tensor_mul(
                                    out=ctxt_s[hh * 64:hh * 64 + 64,
                                               hp * S + qof:
                                               hp * S + qof + 1024],
                                    in0=st_h, in1=rb)
                            yield u_norm
                    pending_norm.append(norm_units())

                # ---- phase 3: output projection (partial: this core's
                # 512 features x Wo rows; host sums the two partials).
                # qc 0-7 read only jq=0 query rows, so the last pair's
                # (hp3/jq1) deferred normalize hides under them; its
                # units are finished before qc 8.
                for qc in range(16):
                    if qc == 8:
                        for gen in pending_norm:
                            finish(gen)
                        pending_norm.clear()
                    elif pending_norm:
                        for u in pending_norm[0]:
                            u()
                            break
                        else:
                            pending_norm.pop(0)
                    pO = psS.tile([128, 1024], dt.float32, tag="sp",
                                  name=f"pO{qc}")
                    for hp in range(HP):
                        lhsT = ctxt_s[:, hp * S + qc * 128:
                                      hp * S + (qc + 1) * 128]
                        for do2 in range(2):
                            nc.tensor.matmul(
                                out=pO[:, do2 * 512:(do2 + 1) * 512],
                                lhsT=lhsT,
                                rhs=wo_s[:, hp * D + do2 * 512:
                                         hp * D + (do2 + 1) * 512],
                                start=(hp == 0), stop=(hp == HP - 1))
                    ot = opool.tile([128, 1024], dt.float32, tag="ot",
                                    name=f"ot{qc}")
                    nc.vector.tensor_add(out=ot, in0=pO, in1=bob)
                    nc.sync.dma_start(
                        out=out[qc * 128:(qc + 1) * 128, :], in_=ot)

    nc.compile()
    return nc


def _make_in_maps(inputs):
    hidden_states = inputs["hidden_states"]
    attention_mask = inputs["attention_mask"]
    Wq = np.asarray(inputs["Wq"])
    Wk = np.asarray(inputs["Wk"])
    Wv = np.asarray(inputs["Wv"])
    Wo = np.asarray(inputs["Wo"])
    radd = (np.asarray(inputs["bv"]).astype(np.float32) @
            Wo.astype(np.float32) +
            np.asarray(inputs["bo"]).astype(np.float32)).astype(np.float32)
    zeros = np.zeros_like(radd)

    in_maps = []
    for c in range(N_CORES):
        b, j = c // 2, c % 2
        sl = slice(j * 512, (j + 1) * 512)
        xt_b = np.ascontiguousarray(np.asarray(hidden_states[b]).T.astype(BF))
        in_maps.append({
            "xt": xt_b,
            "wq": np.ascontiguousarray(Wq[:, sl].astype(BF)),
            "wk": np.ascontiguousarray(Wk[:, sl].astype(BF)),
            "wv": np.ascontiguousarray(Wv[:, sl].astype(BF)),
            "wo": np.ascontiguousarray(Wo[sl, :].astype(BF)),
            "bq": np.ascontiguousarray(
                np.asarray(inputs["bq"])[sl].astype(np.float32)),
            "bk": np.ascontiguousarray(
                np.asarray(inputs["bk"])[sl].astype(np.float32)),
            "radd": np.ascontiguousarray(radd if j == 0 else zeros),
            "mk": np.ascontiguousarray(
                np.asarray(attention_mask[b, 0, 0, :]).astype(np.float32) + NEG_C),
        })
    return in_maps


def _gather(res):
    full = np.empty((B, S, D), dtype=np.float32)
    for b in range(B):
        full[b] = res.results[2 * b]["out"] + res.results[2 * b + 1]["out"]
    return full


def kernel(hidden_states, attention_mask, Wq, bq, Wk, bk, Wv, bv, Wo, bo):
    from concourse.bass_utils import run_bass_kernel_spmd

    nc = _build()
    in_maps = _make_in_maps(dict(
        hidden_states=hidden_states, attention_mask=attention_mask,
        Wq=Wq, bq=bq, Wk=Wk, bk=bk, Wv=Wv, bv=bv, Wo=Wo, bo=bo))
    res = run_bass_kernel_spmd(nc, in_maps, list(range(N_CORES)))
    return _gather(res)


# revision 23
# speedup vs baseline: 1.0286x; 1.0286x over previous
"""Multi-head attention (B=4, S=2048, D=1024, H=16, DK=64) on 8 Trainium2
NeuronCores.

v3: head-sharding.  Core c = 2*b + j handles batch b = c//2 and HEADS
j*8..j*8+7 over the FULL query range (v1/v2 split queries, which made
both cores of a batch recompute full-S K/V — 131k duplicated PE-cycles
per core).  Each core now projects only its 8 heads' K/V/Q columns and
emits a PARTIAL output (its heads' ctx @ its 512 Wo rows); the host
sums the two partials per batch when gathering.  No collectives.

PE tile-packing as v2: the two heads of a pair run concurrently on
disjoint array quadrants (scores on row groups, PV ctx on col groups,
4-way col-packed [1,512] denominator matmuls at partitions 0/32/64/96
of one PSUM bank).  E^T = exp(scores^T/8 + mask - 3) streamed per
(pair, chunk); K/Q projection chains for the next head-pair drip-fed
as PE filler.  bv/bo fold into radd = bv @ Wo + bo host-side (applied
on the j=0 core only).
"""

import itertools

import numpy as np
import ml_dtypes

B, S, D, H, DK = 4, 2048, 1024, 16, 64
N_CORES = 8
HL = 8               # local heads per core
HP = HL // 2         # local head-pairs
NEG_C = -3.0         # exp stabilizer; cancels exactly in normalization
BF = ml_dtypes.bfloat16


def _build():
    import concourse.mybir as mybir
    import concourse.tile as tile
    from concourse import bacc

    dt = mybir.dt
    AF = mybir.ActivationFunctionType
    nc = bacc.Bacc("TRN2", num_devices=N_CORES)

    W = HL * DK      # 512: this core's projection width
    xt = nc.declare_dram_parameter("xt", [D, S], dt.bfloat16, isOutput=False)
    wq = nc.declare_dram_parameter("wq", [D, W], dt.bfloat16, isOutput=False)
    wk = nc.declare_dram_parameter("wk", [D, W], dt.bfloat16, isOutput=False)
    wv = nc.declare_dram_parameter("wv", [D, W], dt.bfloat16, isOutput=False)
    wo = nc.declare_dram_parameter("wo", [W, D], dt.bfloat16, isOutput=False)
    bq = nc.declare_dram_parameter("bq", [W], dt.float32, isOutput=False)
    bk = nc.declare_dram_parameter("bk", [W], dt.float32, isOutput=False)
    radd = nc.declare_dram_parameter("radd", [D], dt.float32, isOutput=False)
    mk = nc.declare_dram_parameter("mk", [S], dt.float32, isOutput=False)
    out = nc.declare_dram_parameter("out", [S, D], dt.float32, isOutput=True)

    with tile.TileContext(nc) as tc:
        with (
            tc.tile_pool(name="pers", bufs=1) as pers,
            tc.tile_pool(name="psS", bufs=2, space="PSUM") as psS,
            tc.tile_pool(name="psF", bufs=1, space="PSUM") as psF,
            tc.tile_pool(name="ctxp", bufs=2, space="PSUM") as ctxp,
            tc.tile_pool(name="denp", bufs=1, space="PSUM") as denp,
            tc.tile_pool(name="ktp", bufs=2) as ktp,
            tc.tile_pool(name="qtp", bufs=2) as qtp,
            tc.tile_pool(name="wstr", bufs=2) as wstr,
        ):
            # ---- persistent SBUF arrays -------------------------------
            v_s = pers.tile([128, 16 * W], dt.bfloat16, tag="v")
            xt_s = pers.tile([128, 8 * S], dt.bfloat16, tag="xt")
            wk_s = pers.tile([128, 8 * W], dt.bfloat16, tag="wk")
            bqc = pers.tile([128, HP], dt.float32, tag="bqc")
            bkc = pers.tile([128, HP], dt.float32, tag="bkc")
            mkc = pers.tile([128, 16], dt.float32, tag="mkc")
            ones_s = pers.tile([128, 32], dt.bfloat16, tag="ones")

            nc.sync.dma_start(out=bqc, in_=bq.rearrange("(a p) -> p a", p=128))
            nc.sync.dma_start(out=bkc, in_=bk.rearrange("(a p) -> p a", p=128))
            nc.sync.dma_start(out=mkc, in_=mk.rearrange("(a p) -> p a", p=128))
            nc.gpsimd.memset(ones_s, 1.0)

            # ---- phase 1: V (wv freed after) --------------------------
            with tc.tile_pool(name="poolA", bufs=1) as poolA:
                wv_s = poolA.tile([128, 8 * W], dt.bfloat16, tag="wv")
                for c in range(8):
                    nc.sync.dma_start(
                        out=xt_s[:, c * S:(c + 1) * S],
                        in_=xt[c * 128:(c + 1) * 128, :])
                    nc.sync.dma_start(
                        out=wv_s[:, c * W:(c + 1) * W],
                        in_=wv[c * 128:(c + 1) * 128, :])
                for c in range(8):
                    nc.sync.dma_start(
                        out=wk_s[:, c * W:(c + 1) * W],
                        in_=wk[c * 128:(c + 1) * 128, :])

                for sc2 in range(8):
                    pv = psS.tile([128, 1024], dt.float32, tag="sp",
                                  name=f"pv{sc2}")
                    for half in range(2):
                        sc = 2 * sc2 + half
                        for c in range(8):
                            nc.tensor.matmul(
                                out=pv[:, half * 512:(half + 1) * 512],
                                lhsT=xt_s[:, c * S + sc * 128:
                                          c * S + (sc + 1) * 128],
                                rhs=wv_s[:, c * W:(c + 1) * W],
                                start=(c == 0), stop=(c == 7))
                    # pv columns are (h, dk) pairs == the V layout we want
                    nc.vector.tensor_copy(
                        v_s[:, sc2 * 1024:(sc2 + 1) * 1024], pv)

            # ---- phase 2: interleaved projections + attention ---------
            with (
                tc.tile_pool(name="attin", bufs=1) as attin,
                tc.tile_pool(name="epool", bufs=10) as epool,
                tc.tile_pool(name="rcpp", bufs=5) as rcpp,
                tc.tile_pool(name="rpool", bufs=3) as rpool,
                tc.tile_pool(name="stg", bufs=2) as stg,
                tc.tile_pool(name="opool", bufs=2) as opool,
            ):
                ctxt_s = attin.tile([128, HP * S], dt.bfloat16, tag="ctxt")
                wo_s = attin.tile([128, HP * D], dt.bfloat16, tag="wo")
                bob = attin.tile([128, D], dt.float32, tag="bob")
                for r in range(HP):
                    nc.sync.dma_start(
                        out=wo_s[:, r * D:(r + 1) * D],
                        in_=wo[r * 128:(r + 1) * 128, :])

                def _bcast_src(ap):
                    import concourse.bass as bass
                    return bass.AP(
                        tensor=ap.tensor, offset=ap.offset,
                        ap=[[0, 128]] + [list(p) for p in ap.ap])

                nc.gpsimd.dma_start(out=bob, in_=_bcast_src(radd[:]))

                kt_tiles = {}
                qt_tiles = {}

                # Filler-unit generators: projection chains for head-pair
                # `hp`, emitted in small steps interleaved with attention.
                def k_chain_units(hp):
                    kt_t = ktp.tile([128, S], dt.bfloat16, tag="ktt",
                                    name=f"ktt{hp}")
                    kt_tiles[hp] = kt_t
                    for t in range(4):
                        pk = psF.tile([128, 512], dt.float32, tag="pf",
                                      name=f"pk{hp}_{t}")
                        for c in range(8):
                            def do_k(c=c, t=t, pk=pk):
                                nc.tensor.matmul(
                                    out=pk,
                                    lhsT=wk_s[:, c * W + hp * 128:
                                              c * W + (hp + 1) * 128],
                                    rhs=xt_s[:, c * S + t * 512:
                                             c * S + (t + 1) * 512],
                                    start=(c == 0), stop=(c == 7))
                            yield do_k
                        def drain_k(t=t, pk=pk, kt_t=kt_t):
                            nc.vector.tensor_scalar_add(
                                kt_t[:, t * 512:(t + 1) * 512],
                                pk, bkc[:, hp:hp + 1])
                        yield drain_k

                def q_chain_units(hp):
                    qt_t = qtp.tile([128, S], dt.bfloat16, tag="qtt",
                                    name=f"qtt{hp}")
                    qt_tiles[hp] = qt_t
                    wqc = wstr.tile([128, 1024], dt.bfloat16, tag="wqs",
                                    name=f"wqc{hp}")
                    nc.sync.dma_start(
                        out=wqc.rearrange("p (c n) -> p c n", n=128),
                        in_=wq.rearrange("(c p) n -> p c n", p=128)[
                            :, :, hp * 128:(hp + 1) * 128])
                    for t in range(4):
                        pq = psF.tile([128, 512], dt.float32, tag="pf",
                                      name=f"pq{hp}_{t}")
                        for c in range(8):
                            def do_q(c=c, t=t, pq=pq, wqc=wqc):
                                nc.tensor.matmul(
                                    out=pq,
                                    lhsT=wqc[:, c * 128:(c + 1) * 128],
                                    rhs=xt_s[:, c * S + t * 512:
                                             c * S + (t + 1) * 512],
                                    start=(c == 0), stop=(c == 7))
                            yield do_q
                        def drain_q(t=t, pq=pq, qt_t=qt_t):
                            nc.vector.tensor_scalar_add(
                                qt_t[:, t * 512:(t + 1) * 512],
                                pq, bqc[:, hp:hp + 1])
                        yield drain_q

                def drive(gen, n=1):
                    if gen is None:
                        return
                    for _ in range(n):
                        for u in gen:
                            u()
                            break
                        else:
                            return

                def finish(gen):
                    if gen is not None:
                        for u in gen:
                            u()

                # prime head-pair 0 (wide PSUM tiles from psS — the
                # quarter-granularity psF bank would bubble on drains
                # with no attention work interleaved yet)
                kt_t = ktp.tile([128, S], dt.bfloat16, tag="ktt",
                                name="ktt0")
                kt_tiles[0] = kt_t
                for hf in range(2):
                    pk = psS.tile([128, 1024], dt.float32, tag="sp",
                                  name=f"Ppk{hf}")
                    for c in range(8):
                        lhsT = wk_s[:, c * W:c * W + 128]
                        for st2 in range(2):
                            nc.tensor.matmul(
                                out=pk[:, st2 * 512:(st2 + 1) * 512],
                                lhsT=lhsT,
                                rhs=xt_s[:, c * S + hf * 1024 + st2 * 512:
                                         c * S + hf * 1024 + (st2 + 1) * 512],
                                start=(c == 0), stop=(c == 7))
                    nc.vector.tensor_scalar_add(
                        kt_t[:, hf * 1024:(hf + 1) * 1024],
                        pk, bkc[:, 0:1])
                qt_t = qtp.tile([128, S], dt.bfloat16, tag="qtt",
                                name="qtt0")
                qt_tiles[0] = qt_t
                wqc = wstr.tile([128, 1024], dt.bfloat16, tag="wqs",
                                name="wqc0")
                nc.sync.dma_start(
                    out=wqc.rearrange("p (c n) -> p c n", n=128),
                    in_=wq.rearrange("(c p) n -> p c n", p=128)[:, :, 0:128])
                for hf in range(2):
                    pq = psS.tile([128, 1024], dt.float32, tag="sp",
                                  name=f"Ppq{hf}")
                    for c in range(8):
                        lhsT = wqc[:, c * 128:(c + 1) * 128]
                        for st2 in range(2):
                            nc.tensor.matmul(
                                out=pq[:, st2 * 512:(st2 + 1) * 512],
                                lhsT=lhsT,
                                rhs=xt_s[:, c * S + hf * 1024 + st2 * 512:
                                         c * S + hf * 1024 + (st2 + 1) * 512],
                                start=(c == 0), stop=(c == 7))
                    nc.vector.tensor_scalar_add(
                        qt_t[:, hf * 1024:(hf + 1) * 1024],
                        pq, bqc[:, 0:1])

                fillers = {
                    hp: itertools.chain(k_chain_units(hp), q_chain_units(hp))
                    for hp in range(1, HP)
                }

                DEN_LAG = 3  # see den-bank release note below
                pending_norm = []

                for vp in range(2 * HP):
                    hp, jq = vp // 2, vp % 2
                    filler = fillers.get(vp // 2 + 1)
                    kt_t, qt_t = kt_tiles[hp], qt_tiles[hp]
                    hA, hB = 2 * hp, 2 * hp + 1
                    qof = jq * 1024
                    # one PSUM bank per q-half holds BOTH heads' ctx
                    cx = [ctxp.tile([128, 512], dt.float32, tag="cx",
                                    name=f"cx{vp}_{q2}") for q2 in range(2)]
                    # one bank holds the 4 denominator accumulators at
                    # partitions 0/32/64/96 (q2*64 + head*32)
                    dn = denp.tile([128, 512], dt.float32, tag="dn",
                                   name=f"dn{vp}")
                    e_hist = {}
                    for g in range(16 + DEN_LAG):
                        if g < 16:
                            scc = g
                            spA = psS.tile([128, 1024], dt.float32, tag="sp",
                                           name=f"spA{vp}_{scc}")
                            spB = psS.tile([128, 1024], dt.float32, tag="sp",
                                           name=f"spB{vp}_{scc}")
                            for po, sp in ((0, spA), (64, spB)):
                                lhsT = kt_t[po:po + 64,
                                            scc * 128:(scc + 1) * 128]
                                for q2 in range(2):
                                    nc.tensor.matmul(
                                        out=sp[:, q2 * 512:(q2 + 1) * 512],
                                        lhsT=lhsT,
                                        rhs=qt_t[po:po + 64,
                                                 qof + q2 * 512:
                                                 qof + (q2 + 1) * 512],
                                        start=True, stop=True)
                            eA = epool.tile([128, 1024], dt.bfloat16,
                                            tag="e", name=f"eA{vp}_{scc}")
                            eB = epool.tile([128, 1024], dt.bfloat16,
                                            tag="e", name=f"eB{vp}_{scc}")
                            nc.scalar.activation(
                                out=eA, in_=spA, func=AF.Exp,
                                bias=mkc[:, scc:scc + 1],
                                scale=1.0 / np.sqrt(DK))
                            nc.scalar.activation(
                                out=eB, in_=spB, func=AF.Exp,
                                bias=mkc[:, scc:scc + 1],
                                scale=1.0 / np.sqrt(DK))
                            e_hist[scc] = (eA, eB)
                        if 1 <= g <= 16:
                            scc = g - 1
                            eA, eB = e_hist[scc]
                            st_, sp_ = (scc == 0), (scc == 15)
                            for q2 in range(2):
                                for po, h, e_ in ((0, hA, eA), (64, hB, eB)):
                                    vh = v_s[:, scc * W + h * DK:
                                             scc * W + (h + 1) * DK]
                                    nc.tensor.matmul(
                                        out=cx[q2][po:po + 64, :], lhsT=vh,
                                        rhs=e_[:, q2 * 512:(q2 + 1) * 512],
                                        start=st_, stop=sp_,
                                        skip_group_check=True)
                        # den + filler emitted in 2-chunk blocks (every
                        # other g) -- fewer PE tiling-mode switches
                        if DEN_LAG <= g and (g % 2 == 1 or g == 16 + DEN_LAG - 1):
                            den_sccs = [s_ for s_ in range(16)
                                        if s_ <= g - DEN_LAG and s_ in e_hist]
                            for scc in den_sccs:
                                eA, eB = e_hist.pop(scc)
                                st_, sp_ = (scc == 0), (scc == 15)
                                for q2 in range(2):
                                    for hh, e_ in ((0, eA), (1, eB)):
                                        p = q2 * 64 + hh * 32
                                        nc.tensor.matmul(
                                            out=dn[p:p + 32, :], lhsT=ones_s,
                                            rhs=e_[:, q2 * 512:(q2 + 1) * 512],
                                            start=st_, stop=sp_,
                                            tile_position=(0, p),
                                            skip_group_check=True)
                        if pending_norm:
                            for u in pending_norm[0]:
                                u()
                                break
                            else:
                                pending_norm.pop(0)
                        if g % 2 == 0:
                            drive(filler, 4)
                    if vp % 2 == 1:
                        finish(filler)

                    # drain ctx + den: fast copies PSUM->SBUF free the
                    # banks within ~1us of the last matmul.  Head B's rows
                    # move from PSUM partitions 64-127 to a base-0 tile so
                    # the muls see matching input base partitions.
                    stA = stg.tile([64, 1024], dt.bfloat16, tag="stA",
                                   name=f"stA{vp}")
                    stB = stg.tile([64, 1024], dt.bfloat16, tag="stB",
                                   name=f"stB{vp}")
                    for q2 in range(2):
                        nc.vector.tensor_copy(
                            stA[:, q2 * 512:(q2 + 1) * 512], cx[q2][0:64, :])
                        nc.vector.tensor_copy(
                            stB[:, q2 * 512:(q2 + 1) * 512], cx[q2][64:128, :])
                    dnc = stg.tile([128, 512], dt.float32, tag="dnc",
                                   name=f"dnc{vp}")
                    nc.vector.tensor_copy(dnc, dn)

                    # the slow normalize chain (one merged reciprocal,
                    # per-accumulator source rows, broadcasts, muls) is
                    # deferred into the next pair's g-steps so it never
                    # blocks filler drains in the DVE FIFO
                    def norm_units(vp=vp, hp=hp, qof=qof,
                                   stA=stA, stB=stB, dnc=dnc):
                        rcp_s = rcpp.tile([128, 512], dt.bfloat16,
                                          tag="rcp", name=f"rcp{vp}")
                        def u_recip():
                            with nc.allow_low_precision(
                                    reason="softmax denom ~1e2, bf16 ok"):
                                nc.vector.reciprocal(out=rcp_s, in_=dnc)
                        yield u_recip
                        srcs = {}
                        for q2 in range(2):
                            for hh in range(2):
                                p = q2 * 64 + hh * 32
                                r = rcpp.tile([1, 512], dt.bfloat16,
                                              tag="rsrc",
                                              name=f"rsrc{vp}_{q2}_{hh}")
                                srcs[(q2, hh)] = r
                                def u_src(r=r, p=p):
                                    nc.vector.tensor_copy(
                                        r, rcp_s[p:p + 1, :])
                                yield u_src
                        for hh, st_h in ((0, stA), (1, stB)):
                            rb = rpool.tile([64, 1024], dt.bfloat16,
                                            tag="rb", name=f"rb{vp}_{hh}")
                            def u_norm(hh=hh, st_h=st_h, rb=rb):
                                for q2 in range(2):
                                    nc.gpsimd.partition_broadcast(
                                        rb[:, q2 * 512:(q2 + 1) * 512],
                                        srcs[(q2, hh)][0:1, :])
                                nc.vector.tensor_mul(
                                    out=ctxt_s[hh * 64:hh * 64 + 64,
                                               hp * S + qof:
                                               hp * S + qof + 1024],
                                    in0=st_h, in1=rb)
                            yield u_norm
                    pending_norm.append(norm_units())

                for gen in pending_norm:
                    finish(gen)
                pending_norm.clear()

                # ---- phase 3: output projection (partial: this core's
                # 512 features x Wo rows; host sums the two partials) ---
                for qc in range(16):
                    pO = psS.tile([128, 1024], dt.float32, tag="sp",
                                  name=f"pO{qc}")
                    for hp in range(HP):
                        lhsT = ctxt_s[:, hp * S + qc * 128:
                                      hp * S + (qc + 1) * 128]
                        for do2 in range(2):
                            nc.tensor.matmul(
                                out=pO[:, do2 * 512:(do2 + 1) * 512],
                                lhsT=lhsT,
                                rhs=wo_s[:, hp * D + do2 * 512:
                                         hp * D + (do2 + 1) * 512],
                                start=(hp == 0), stop=(hp == HP - 1))
                    ot = opool.tile([128, 1024], dt.float32, tag="ot",
                                    name=f"ot{qc}")
                    nc.vector.tensor_add(out=ot, in0=pO, in1=bob)
                    nc.sync.dma_start(
                        out=out[qc * 128:(qc + 1) * 128, :], in_=ot)

    nc.compile()
    return nc


def _make_in_maps(inputs):
    hidden_states = inputs["hidden_states"]
    attention_mask = inputs["attention_mask"]
    Wq = np.asarray(inputs["Wq"])
    Wk = np.asarray(inputs["Wk"])
    Wv = np.asarray(inputs["Wv"])
    Wo = np.asarray(inputs["Wo"])
    radd = (np.asarray(inputs["bv"]).astype(np.float32) @
            Wo.astype(np.float32) +
            np.asarray(inputs["bo"]).astype(np.float32)).astype(np.float32)
    zeros = np.zeros_like(radd)

    in_maps = []
    for c in range(N_CORES):
        b, j = c // 2, c % 2
        sl = slice(j * 512, (j + 1) * 512)
        xt_b = np.ascontiguousarray(np.asarray(hidden_states[b]).T.astype(BF))
        in_maps.append({
            "xt": xt_b,
            "wq": np.ascontiguousarray(Wq[:, sl].astype(BF)),
            "wk": np.ascontiguousarray(Wk[:, sl].astype(BF)),
            "wv": np.ascontiguousarray(Wv[:, sl].astype(BF)),
            "wo": np.ascontiguousarray(Wo[sl, :].astype(BF)),
            "bq": np.ascontiguousarray(
                np.asarray(inputs["bq"])[sl].astype(np.float32)),
            "bk": np.ascontiguousarray(
                np.asarray(inputs["bk"])[sl].astype(np.float32)),
            "radd": np.ascontiguousarray(radd if j == 0 else zeros),
            "mk": np.ascontiguousarray(
                np.asarray(attention_mask[b, 0, 0, :]).astype(np.float32) + NEG_C),
        })
    return in_maps


def _gather(res):
    full = np.empty((B, S, D), dtype=np.float32)
    for b in range(B):
        full[b] = res.results[2 * b]["out"] + res.results[2 * b + 1]["out"]
    return full


def kernel(hidden_states, attention_mask, Wq, bq, Wk, bk, Wv, bv, Wo, bo):
    from concourse.bass_utils import run_bass_kernel_spmd

    nc = _build()
    in_maps = _make_in_maps(dict(
        hidden_states=hidden_states, attention_mask=attention_mask,
        Wq=Wq, bq=bq, Wk=Wk, bk=bk, Wv=Wv, bv=bv, Wo=Wo, bo=bo))
    res = run_bass_kernel_spmd(nc, in_maps, list(range(N_CORES)))
    return _gather(res)


# revision 25
# speedup vs baseline: 1.1648x; 1.1324x over previous
import itertools
"""Multi-head attention (B=4, S=2048, D=1024, H=16, DK=64) on 8 Trainium2
NeuronCores.

Sharding (head-parallel): core c = 2*b + j handles batch b = c//2 and
HEADS j*8..j*8+7 over the FULL query range.  Each core projects only its
8 heads' K/V/Q columns and emits a PARTIAL output (its heads' ctx @ its
512 Wo rows); the host sums the two partials per batch.  No duplicated
K/V work, no collectives.

Key scheduling idea: the Scalar engine's exp stream (~294us of work)
and the Tensor engine (~383us of matmul work) must both stay busy.
K^T/Q^T projection chains for pair i+1 are statically interleaved as
PE filler between the attention chunks of pair i, so the Tensor engine
never idles long enough for the HAM clock gate to re-throttle it.

Layouts (feature-on-partition for everything left of the softmax):
  X^T [D,S] resident (bf16);  V [S, H*(DK+1)] resident with a ones
  column per head (PV row 64 = softmax denominator);  K^T/Q^T live in
  per-pair streaming tiles;  E^T = exp(scores^T/8 + mask - 3) streamed
  per (head, s-chunk);  ctx^T accumulates; out = ctx^T-as-lhsT @ Wo.
All matmuls bf16 (1 cycle/row), fp32 PSUM, fp32 output.
bv/bo are folded on the host: radd = bv @ Wo + bo (softmax rows sum
to 1, so attn @ (V + bv) @ Wo + bo == attn@V@Wo + radd).
"""

import numpy as np
import ml_dtypes

B, S, D, H, DK = 4, 2048, 1024, 16, 64
HL, HP = 8, 4        # local heads / head-pairs per core
SQ = S               # full query range per core (head-sharded)
W = HL * DK          # 512: this core's projection width
N_CORES = 8
SH = DK + 1          # per-head V width incl. ones column
NEG_C = -3.0         # exp stabilizer; cancels exactly in normalization
BF = ml_dtypes.bfloat16


def _build():
    import concourse.mybir as mybir
    import concourse.tile as tile
    from concourse import bacc

    dt = mybir.dt
    AF = mybir.ActivationFunctionType
    nc = bacc.Bacc("TRN2", num_devices=N_CORES)

    xt = nc.declare_dram_parameter("xt", [D, S], dt.bfloat16, isOutput=False)
    wq = nc.declare_dram_parameter("wq", [D, W], dt.bfloat16, isOutput=False)
    wk = nc.declare_dram_parameter("wk", [D, W], dt.bfloat16, isOutput=False)
    wv = nc.declare_dram_parameter("wv", [D, W], dt.bfloat16, isOutput=False)
    wo = nc.declare_dram_parameter("wo", [W, D], dt.bfloat16, isOutput=False)
    bq = nc.declare_dram_parameter("bq", [W], dt.float32, isOutput=False)
    bk = nc.declare_dram_parameter("bk", [W], dt.float32, isOutput=False)
    radd = nc.declare_dram_parameter("radd", [D], dt.float32, isOutput=False)
    mk = nc.declare_dram_parameter("mk", [S], dt.float32, isOutput=False)
    out = nc.declare_dram_parameter("out", [S, D], dt.float32, isOutput=True)

    with tile.TileContext(nc) as tc:
        with (
            tc.tile_pool(name="pers", bufs=1) as pers,
            tc.tile_pool(name="ps", bufs=3, space="PSUM") as ps,
            tc.tile_pool(name="ctxp", bufs=2, space="PSUM") as ctxp,
            tc.tile_pool(name="ktp", bufs=2) as ktp,
            tc.tile_pool(name="qtp", bufs=2) as qtp,
            tc.tile_pool(name="wstr", bufs=2) as wstr,
        ):
            # ---- persistent SBUF arrays -------------------------------
            v_s = pers.tile([128, 16 * HL * SH], dt.bfloat16, tag="v")
            xt_s = pers.tile([128, 8 * S], dt.bfloat16, tag="xt")
            wk_s = pers.tile([128, 8 * W], dt.bfloat16, tag="wk")
            bqc = pers.tile([128, HP], dt.float32, tag="bqc")
            bkc = pers.tile([128, HP], dt.float32, tag="bkc")
            mkc = pers.tile([128, 16], dt.float32, tag="mkc")

            nc.sync.dma_start(out=bqc, in_=bq.rearrange("(a p) -> p a", p=128))
            nc.sync.dma_start(out=bkc, in_=bk.rearrange("(a p) -> p a", p=128))
            nc.sync.dma_start(out=mkc, in_=mk.rearrange("(a p) -> p a", p=128))

            # ---- phase 1: V (wv freed after) --------------------------
            with tc.tile_pool(name="poolA", bufs=1) as poolA:
                wv_s = poolA.tile([128, 8 * W], dt.bfloat16, tag="wv")
                for c in range(8):
                    nc.sync.dma_start(
                        out=xt_s[:, c * S:(c + 1) * S],
                        in_=xt[c * 128:(c + 1) * 128, :])
                    nc.sync.dma_start(
                        out=wv_s[:, c * W:(c + 1) * W],
                        in_=wv[c * 128:(c + 1) * 128, :])
                for c in range(8):
                    nc.sync.dma_start(
                        out=wk_s[:, c * W:(c + 1) * W],
                        in_=wk[c * 128:(c + 1) * 128, :])

                for sc2 in range(8):
                    pv = ps.tile([128, 1024], dt.float32, tag="ps",
                                 name=f"pv{sc2}")
                    for half in range(2):
                        sc = 2 * sc2 + half
                        for c in range(8):
                            nc.tensor.matmul(
                                out=pv[:, half * 512:(half + 1) * 512],
                                lhsT=xt_s[:, c * S + sc * 128:
                                          c * S + (sc + 1) * 128],
                                rhs=wv_s[:, c * W:(c + 1) * W],
                                start=(c == 0), stop=(c == 7))
                    for half in range(2):
                        sc = 2 * sc2 + half
                        v3 = v_s[:, sc * HL * SH:(sc + 1) * HL * SH].rearrange(
                            "p (h e) -> p h e", e=SH)
                        nc.gpsimd.memset(v3[:, :, DK:SH], 1.0)
                        nc.vector.tensor_copy(
                            v3[:, :, 0:DK],
                            pv[:, half * 512:(half + 1) * 512].rearrange(
                                "p (h d) -> p h d", d=DK))

            # ---- phase 2: interleaved projections + attention ---------
            with (
                tc.tile_pool(name="attin", bufs=1) as attin,
                tc.tile_pool(name="epool", bufs=8) as epool,
                tc.tile_pool(name="rpool", bufs=1) as rpool,
                tc.tile_pool(name="stg", bufs=2) as stg,
                tc.tile_pool(name="opool", bufs=2) as opool,
            ):
                ctxt_s = attin.tile([128, HP * S], dt.bfloat16, tag="ctxt")
                wo_s = attin.tile([128, HP * D], dt.bfloat16, tag="wo")
                bob = attin.tile([128, D], dt.float32, tag="bob")
                for r in range(HP):
                    nc.sync.dma_start(
                        out=wo_s[:, r * D:(r + 1) * D],
                        in_=wo[r * 128:(r + 1) * 128, :])

                def _bcast_src(ap):
                    import concourse.bass as bass
                    return bass.AP(
                        tensor=ap.tensor, offset=ap.offset,
                        ap=[[0, 128]] + [list(p) for p in ap.ap])

                nc.gpsimd.dma_start(out=bob, in_=_bcast_src(radd[:]))

                kt_tiles = {}
                qt_tiles = {}

                # Filler-unit generators: emit projection chains for pair
                # `i` in small steps so they interleave with attention.
                def k_chain_units(i):
                    kt_t = ktp.tile([128, S], dt.bfloat16, tag="ktt",
                                    name=f"ktt{i}")
                    kt_tiles[i] = kt_t
                    for hf in range(2):
                        pk = ps.tile([128, 1024], dt.float32, tag="ps",
                                        name=f"pk{i}_{hf}")
                        for c in range(8):
                            lhsT = wk_s[:, c * W + i * 128: c * W + (i + 1) * 128]
                            def do_k(c=c, hf=hf, pk=pk, lhsT=lhsT):
                                for st in range(2):
                                    nc.tensor.matmul(
                                        out=pk[:, st * 512:(st + 1) * 512],
                                        lhsT=lhsT,
                                        rhs=xt_s[:, c * S + hf * 1024 + st * 512:
                                                 c * S + hf * 1024 + (st + 1) * 512],
                                        start=(c == 0), stop=(c == 7))
                            yield do_k
                        def drain_k(hf=hf, pk=pk, kt_t=kt_t):
                            nc.vector.tensor_scalar_add(
                                kt_t[:, hf * 1024:(hf + 1) * 1024],
                                pk, bkc[:, i:i + 1])
                        yield drain_k

                def q_chain_units(i):
                    qt_t = qtp.tile([128, S], dt.bfloat16, tag="qtt",
                                    name=f"qtt{i}")
                    qt_tiles[i] = qt_t
                    wqc = wstr.tile([128, 1024], dt.bfloat16, tag="wqs",
                                    name=f"wqc{i}")
                    nc.sync.dma_start(
                        out=wqc.rearrange("p (c n) -> p c n", n=128),
                        in_=wq.rearrange("(c p) n -> p c n", p=128)[
                            :, :, i * 128:(i + 1) * 128])
                    for hf in range(2):
                        pq = ps.tile([128, 1024], dt.float32, tag="ps",
                                     name=f"pq{i}_{hf}")
                        for c in range(8):
                            def do_q(c=c, hf=hf, pq=pq, wqc=wqc):
                                lhsT = wqc[:, c * 128:(c + 1) * 128]
                                for q2 in range(2):
                                    nc.tensor.matmul(
                                        out=pq[:, q2 * 512:(q2 + 1) * 512],
                                        lhsT=lhsT,
                                        rhs=xt_s[:, c * S + hf * 1024 + q2 * 512:
                                                 c * S + hf * 1024 + (q2 + 1) * 512],
                                        start=(c == 0), stop=(c == 7))
                            yield do_q
                        def drain_q(hf=hf, pq=pq, qt_t=qt_t):
                            nc.vector.tensor_scalar_add(
                                qt_t[:, hf * 1024:(hf + 1) * 1024],
                                pq, bqc[:, i:i + 1])
                        yield drain_q

                def drive(gen, n=1):
                    if gen is None:
                        return
                    for _ in range(n):
                        for u in gen:
                            u()
                            break
                        else:
                            return

                def finish(gen):
                    if gen is not None:
                        for u in gen:
                            u()

                # prime head-pair 0 (used by the first two virtual pairs)
                finish(k_chain_units(0))
                finish(q_chain_units(0))
                fillers = {
                    hp: itertools.chain(k_chain_units(hp), q_chain_units(hp))
                    for hp in range(1, HP)
                }

                for vp in range(2 * HP):
                    hp, jq = vp // 2, vp % 2
                    qof = jq * 1024
                    filler = fillers.get(vp // 2 + 1)
                    kt_t, qt_t = kt_tiles[hp], qt_tiles[hp]
                    for hx in range(2):
                        h = 2 * hp + hx
                        po = hx * 64
                        cx = [ctxp.tile([SH, 512], dt.float32, tag="cx",
                                        name=f"cx{vp}_{h}_{q2}") for q2 in range(2)]
                        # two chunks per iteration: 4-MM same-class runs
                        # hide LDWEIGHTS switches; PV one iteration behind
                        e_hist = {}
                        for g in range(9):
                            if g < 8:
                                for scc in (2 * g, 2 * g + 1):
                                    sp = ps.tile([128, 1024], dt.float32,
                                                 tag="ps", name=f"sp{vp}_{h}_{scc}")
                                    lhsT = kt_t[po:po + 64,
                                                scc * 128:(scc + 1) * 128]
                                    for q2 in range(2):
                                        nc.tensor.matmul(
                                            out=sp[:, q2 * 512:(q2 + 1) * 512],
                                            lhsT=lhsT,
                                            rhs=qt_t[po:po + 64,
                                                     qof + q2 * 512:
                                                     qof + (q2 + 1) * 512],
                                            start=True, stop=True)
                                    e = epool.tile([128, 1024], dt.bfloat16,
                                                   tag="e", name=f"e{vp}_{h}_{scc}")
                                    nc.scalar.activation(
                                        out=e, in_=sp, func=AF.Exp,
                                        bias=mkc[:, scc:scc + 1],
                                        scale=1.0 / np.sqrt(DK))
                                    e_hist[scc] = e
                            if g > 0:
                                for scc in (2 * g - 2, 2 * g - 1):
                                    vh = v_s[:, scc * HL * SH + h * SH:
                                             scc * HL * SH + (h + 1) * SH]
                                    for q2 in range(2):
                                        nc.tensor.matmul(
                                            out=cx[q2], lhsT=vh,
                                            rhs=e_hist[scc][:, q2 * 512:(q2 + 1) * 512],
                                            start=(scc == 0), stop=(scc == 15))
                                    del e_hist[scc]
                            drive(filler, 1)
                        # drain ctx: copy PSUM->SBUF fast (frees cx slots),
                        # then normalize at leisure
                        st_t = stg.tile([SH, 1024], dt.float32, tag="stg",
                                        name=f"stg{vp}_{h}")
                        for q2 in range(2):
                            nc.vector.tensor_copy(
                                st_t[:, q2 * 512:(q2 + 1) * 512], cx[q2])
                        rcp = rpool.tile([1, 1024], dt.float32, tag="rcp",
                                         name=f"rcp{vp}_{h}")
                        nc.vector.reciprocal(out=rcp, in_=st_t[DK:SH, :])
                        rb = rpool.tile([64, 1024], dt.float32, tag="rb",
                                        name=f"rb{vp}_{h}")
                        nc.gpsimd.partition_broadcast(rb, rcp[0:1, :])
                        nc.vector.tensor_mul(
                            out=ctxt_s[po:po + 64,
                                       hp * S + qof:hp * S + qof + 1024],
                            in0=st_t[0:DK, :], in1=rb)
                    if vp % 2 == 1:
                        finish(filler)

                # ---- phase 3: output projection -----------------------
                for qc in range(16):
                    pO = ps.tile([128, 1024], dt.float32, tag="ps",
                                 name=f"pO{qc}")
                    for hp in range(HP):
                        lhsT = ctxt_s[:, hp * S + qc * 128: hp * S + (qc + 1) * 128]
                        for do2 in range(2):
                            nc.tensor.matmul(
                                out=pO[:, do2 * 512:(do2 + 1) * 512],
                                lhsT=lhsT,
                                rhs=wo_s[:, hp * D + do2 * 512: hp * D + (do2 + 1) * 512],
                                start=(hp == 0), stop=(hp == HP - 1))
                    ot = opool.tile([128, 1024], dt.float32, tag="ot",
                                    name=f"ot{qc}")
                    nc.vector.tensor_add(out=ot, in0=pO, in1=bob)
                    nc.sync.dma_start(
                        out=out[qc * 128:(qc + 1) * 128, :], in_=ot)

    nc.compile()
    return nc


def _make_in_maps(inputs):
    hidden_states = inputs["hidden_states"]
    attention_mask = inputs["attention_mask"]
    Wq = np.asarray(inputs["Wq"])
    Wk = np.asarray(inputs["Wk"])
    Wv = np.asarray(inputs["Wv"])
    Wo = np.asarray(inputs["Wo"])
    radd = (np.asarray(inputs["bv"]).astype(np.float32) @
            Wo.astype(np.float32) +
            np.asarray(inputs["bo"]).astype(np.float32)).astype(np.float32)
    zeros = np.zeros_like(radd)

    in_maps = []
    for c in range(N_CORES):
        b, j = c // 2, c % 2
        sl = slice(j * 512, (j + 1) * 512)
        xt_b = np.ascontiguousarray(np.asarray(hidden_states[b]).T.astype(BF))
        in_maps.append({
            "xt": xt_b,
            "wq": np.ascontiguousarray(Wq[:, sl].astype(BF)),
            "wk": np.ascontiguousarray(Wk[:, sl].astype(BF)),
            "wv": np.ascontiguousarray(Wv[:, sl].astype(BF)),
            "wo": np.ascontiguousarray(Wo[sl, :].astype(BF)),
            "bq": np.ascontiguousarray(
                np.asarray(inputs["bq"])[sl].astype(np.float32)),
            "bk": np.ascontiguousarray(
                np.asarray(inputs["bk"])[sl].astype(np.float32)),
            "radd": np.ascontiguousarray(radd if j == 0 else zeros),
            "mk": np.ascontiguousarray(
                np.asarray(attention_mask[b, 0, 0, :]).astype(np.float32) + NEG_C),
        })
    return in_maps


def _gather(res):
    full = np.empty((B, S, D), dtype=np.float32)
    for b in range(B):
        full[b] = res.results[2 * b]["out"] + res.results[2 * b + 1]["out"]
    return full


def kernel(hidden_states, attention_mask, Wq, bq, Wk, bk, Wv, bv, Wo, bo):
    from concourse.bass_utils import run_bass_kernel_spmd

    nc = _build()
    in_maps = _make_in_maps(dict(
        hidden_states=hidden_states, attention_mask=attention_mask,
        Wq=Wq, bq=bq, Wk=Wk, bk=bk, Wv=Wv, bv=bv, Wo=Wo, bo=bo))
    res = run_bass_kernel_spmd(nc, in_maps, list(range(N_CORES)))
    return _gather(res)
